# revision 20
# baseline (speedup 1.0000x reference)
"""MultiHeadLatentAttention on 8 Trainium2 NeuronCores (Bass/Tile, SPMD).

Sharding (tensor parallel over heads, per the hint, plus refinements):
  - 16 heads / 8 cores = 2 heads per core: q_proj + kv_b_proj output dims and
    o_proj input dim sharded by head.
  - kv_a_proj + rms-norm are token-sharded (512 tokens/core) with an
    AllGather of the normalized latent instead of replicating the kv_a matmul.
  - AllToAll of the attention outputs token-shards the o_proj: each core
    computes the full o_proj for 512 tokens and outputs its token slice.

Perf notes (v2):
  - All HBM-resident operands are bf16 (same 1 cycle/row PE rate as fp32r,
    half the DMA bytes); all matmul accumulation is fp32 in PSUM.
  - Softmax denominator: exp chunks are pair/quad-summed on the DVE (bf16,
    2x mode), so the PE does only 4 ones-matmuls per 512-token block
    instead of 16.  Partition broadcasts (1/den, 1/std) run on the idle
    GpSimd engine instead of PE ones-matmuls.
  - V is produced directly in [tok, d] layout (latent as stationary), so
    the per-chunk PE transposes are gone.
  - o_proj is split by batch and its weights are prefetched during batch-1
    attention, so the PE never idles on the AllToAll tail.
"""

import math
from contextlib import ExitStack

import numpy as np

B, S = 2, 2048
T = B * S                     # 4096 flattened tokens
HID = 2048
H, D = 16, 128
RANK, ROPE = 512, 64
MAX_POS, ORIG_POS = 131072, 8192
BASE = 500000.0
BETA_FAST, BETA_SLOW = 32.0, 1.0
EPS = 1e-6
NCORES = 8
HPC = H // NCORES             # 2 heads per core
TPC = T // NCORES             # 512 tokens per core (kv_a shard)
SPC = S // NCORES             # 256 tokens per (core, batch) after AllToAll

_CACHE: dict = {}


def _yarn_cos_sin():
    """cos/sin tables matching reference.py's yarn_cos_sin (mscale folded)."""
    scaling = MAX_POS / ORIG_POS
    pos_freqs = BASE ** (np.arange(0, ROPE, 2, dtype=np.float64) / ROPE)
    extrap = 1.0 / pos_freqs
    interp = 1.0 / (scaling * pos_freqs)
    low = max(math.floor(ROPE * math.log(ORIG_POS / (BETA_FAST * 2 * math.pi))
                         / (2 * math.log(BASE))), 0)
    high = min(math.ceil(ROPE * math.log(ORIG_POS / (BETA_SLOW * 2 * math.pi))
                         / (2 * math.log(BASE))), ROPE - 1)
    i = np.arange(ROPE // 2, dtype=np.float64)
    smooth = np.clip((i - low) / max(high - low, 1), 0.0, 1.0)
    inv_freq = ((1.0 - smooth) * interp + smooth * extrap).astype(np.float32)
    pos = np.arange(S, dtype=np.float32)
    freqs = pos[:, None] * inv_freq[None, :]              # [S, 32]
    emb = np.concatenate([freqs, freqs], axis=-1)         # [S, 64]
    mscale = 0.1 * math.log(scaling) + 1.0
    cos = (np.cos(emb) * mscale).astype(np.float32)
    sin = (np.sin(emb) * mscale).astype(np.float32)
    return cos.T.copy(), sin.T.copy()                     # [64, S] each


def build_nc(passes=1, sim_mode=False):
    """Build + compile the (single, SPMD) Bass program for all 8 cores."""
    import concourse.tile as tile
    import concourse.mybir as mybir
    from concourse import bacc

    F32 = mybir.dt.float32
    BF = mybir.dt.bfloat16
    AF = mybir.ActivationFunctionType
    RG = [list(range(NCORES))]

    nc = bacc.Bacc("TRN2", target_bir_lowering=False, debug=False,
                   num_devices=1 if sim_mode else NCORES)

    # ---- kernel I/O ----
    hsT_in = nc.dram_tensor("hsT", [HID, T], BF, kind="ExternalInput").ap()
    hsmy_in = nc.dram_tensor("hsmy", [HID, TPC], BF, kind="ExternalInput").ap()
    qwT_in = nc.dram_tensor("qwT", [HID, HPC * D], BF, kind="ExternalInput").ap()
    kvaT_in = nc.dram_tensor("kvaT", [HID, RANK], BF, kind="ExternalInput").ap()
    kvbT_in = nc.dram_tensor("kvbT", [RANK, HPC * 2 * D], BF, kind="ExternalInput").ap()
    owt_in = nc.dram_tensor("owt", [16, 128, HID], BF, kind="ExternalInput").ap()
    cos_in = nc.dram_tensor("cos", [ROPE, S], BF, kind="ExternalInput").ap()
    sinsh_in = nc.dram_tensor("sinsh", [ROPE, S], BF, kind="ExternalInput").ap()
    onesb_in = nc.dram_tensor("onesb", [128, 1], BF, kind="ExternalInput").ap()
    outTs = [nc.dram_tensor(f"outT{p}" if p else "outT", [HID, 2 * SPC], F32,
                            kind="ExternalOutput").ap() for p in range(passes)]

    NH = HID // 128   # 16 hid chunks
    NR = RANK // 128  # 4 rank chunks

    with tile.TileContext(nc) as tc, ExitStack() as ctx0:
        const = ctx0.enter_context(tc.tile_pool(name="const", bufs=1))
        dram = ctx0.enter_context(tc.tile_pool(name="dram", bufs=1, space="DRAM"))

        ones_b = const.tile([128, 1], BF)
        cosb = const.tile([ROPE, S], BF)
        sinsh = const.tile([ROPE, S], BF)
        eps_t = const.tile([1, 1], F32)
        nc.sync.dma_start(ones_b[:], onesb_in[:])
        nc.vector.memset(eps_t[:], EPS)

        for p_ in range(passes):
            # collective bounce buffers
            ag_in = [dram.tile([RANK // 2, TPC], BF, name=f"agin{p_}{h}")
                     for h in range(2)]
            ag_out = [dram.tile([NCORES, RANK // 2, TPC], BF,
                                addr_space="Local" if sim_mode else "Shared",
                                name=f"agout{p_}{h}") for h in range(2)]
            a2a_in = [dram.tile([NCORES, HPC * D, SPC], BF, name=f"a2ain{p_}{b}")
                      for b in range(B)]
            a2a_out = [dram.tile([NCORES, HPC * D, SPC], BF, name=f"a2aout{p_}{b}")
                       for b in range(B)]

            ctx_pass = ExitStack()
            afp = ctx_pass.enter_context(tc.tile_pool(name=f"afp_{p_}", bufs=1))
            af = afp.tile([128, NH * 2 * SPC], BF, name=f"af{p_}")
            owt_sb = afp.tile([128, NH * HID], BF, name=f"owt{p_}")
            qw_sb = afp.tile([128, NH * HPC * D], BF, name=f"qw{p_}")
            kvbT_sb = afp.tile([128, NR * HPC * 2 * D], BF, name=f"kvb{p_}")
            with ExitStack() as ctx_big:
                big = ctx_big.enter_context(tc.tile_pool(name=f"big_{p_}", bufs=1))
                rope_pool = ctx_big.enter_context(
                    tc.tile_pool(name=f"rope_{p_}", bufs=1))

                def rope_block(X):
                    tmp = rope_pool.tile([ROPE, S], BF, tag="rtmp", bufs=1,
                                         name="rtmp")
                    m2 = rope_pool.tile([ROPE, S], BF, tag="rm2", bufs=1,
                                        name="rm2")
                    nc.vector.tensor_mul(tmp[:], X[0:ROPE], cosb[:])
                    nc.vector.tensor_mul(m2[0:32], X[32:64], sinsh[32:64])
                    nc.vector.tensor_mul(m2[32:64], X[0:32], sinsh[0:32])
                    nc.vector.tensor_add(X[0:ROPE], tmp[:], m2[:])

                # per (head j, batch b) tiles, [128, S] each
                qT = [[big.tile([128, S], BF, name=f"qT{p_}{j}{b}") for b in range(B)]
                      for j in range(HPC)]
                kT = [[big.tile([128, S], BF, name=f"kT{p_}{j}{b}") for b in range(B)]
                      for j in range(HPC)]
                vnat = [[big.tile([128, S], BF, name=f"vn{p_}{j}{b}") for b in range(B)]
                        for j in range(HPC)]

                # ---------- P1+P2 fused: q_proj over 4 token groups; kv_a on my
                # 512-token shard rides along with group 0 (shared DMA stream);
                # the rms-norm epilogue + AllGather are deferred to after
                # group 1 so the PE never waits on DVE/Act.
                with ExitStack() as c2:
                    p2 = c2.enter_context(tc.tile_pool(name=f"p2_{p_}", bufs=1))
                    p2ps = c2.enter_context(tc.tile_pool(name=f"p2ps_{p_}", bufs=1, space="PSUM"))
                    ps_lat = [p2ps.tile([128, TPC], F32, name=f"pslat{p_}{m}", tag=f"lat{m}")
                              for m in range(NR)]
                    lat_sb = p2.tile([128, NR * TPC], F32)

                    for g in range(4):            # 1024-token groups
                        b, half = g // 2, g % 2
                        psq = [[p2ps.tile([128, 512], F32, name=f"psq{p_}{g}{m}{t2}",
                                          tag="psq", bufs=4)
                                for t2 in range(2)] for m in range(HPC)]
                        for k in range(NH):
                            if g == 0:
                                nc.sync.dma_start(
                                    qw_sb[:, k * HPC * D:(k + 1) * HPC * D],
                                    qwT_in[k * 128:(k + 1) * 128, :])
                            ht = p2.tile([128, 1024], BF, tag="hsq", bufs=12)
                            nc.sync.dma_start(
                                ht[:], hsT_in[k * 128:(k + 1) * 128,
                                              g * 1024:(g + 1) * 1024])
                            if g == 0:
                                kva_t = p2.tile([128, RANK], BF, tag="kvat", bufs=3)
                                nc.sync.dma_start(
                                    kva_t[:], kvaT_in[k * 128:(k + 1) * 128, :])
                                hm = p2.tile([128, TPC], BF, tag="hsmy", bufs=3)
                                nc.sync.dma_start(
                                    hm[:], hsmy_in[k * 128:(k + 1) * 128, :])
                            for m in range(HPC):
                                for t2 in range(2):
                                    nc.tensor.matmul(
                                        psq[m][t2][:],
                                        qw_sb[:, k * HPC * D + m * 128:
                                              k * HPC * D + (m + 1) * 128],
                                        ht[:, t2 * 512:(t2 + 1) * 512],
                                        start=(k == 0), stop=(k == NH - 1))
                            if g == 0:
                                for m in range(NR):
                                    nc.tensor.matmul(
                                        ps_lat[m][:],
                                        kva_t[:, m * 128:(m + 1) * 128],
                                        hm[:], start=(k == 0), stop=(k == NH - 1))
                        for m in range(HPC):
                            for t2 in range(2):
                                col = half * 1024 + t2 * 512
                                nc.any.tensor_copy(qT[m][b][:, col:col + 512],
                                                   psq[m][t2][:])
                        if half == 1:
                            for j in range(HPC):
                                rope_block(qT[j][b])
                        if g == 0:
                            # drain the latent out of PSUM; square on DVE while
                            # group 1 streams
                            for m in range(NR):
                                nc.any.tensor_copy(
                                    lat_sb[:, m * TPC:(m + 1) * TPC], ps_lat[m][:])
                            sq = [p2.tile([128, TPC], BF, name=f"sq{p_}{m}",
                                          tag="sq", bufs=2) for m in range(NR)]
                            for m in range(NR):
                                nc.vector.tensor_mul(
                                    sq[m][:], lat_sb[:, m * TPC:(m + 1) * TPC],
                                    lat_sb[:, m * TPC:(m + 1) * TPC])
                            nc.sync.dma_start(cosb[:], cos_in[:])
                            nc.sync.dma_start(sinsh[:], sinsh_in[:])
                        if g == 1:
                            # rms-norm stats; var accumulates into a corner
                            # of the (drained) ps_lat[0] bank
                            ps_var = ps_lat[0][0:1, :]
                            for m in range(NR):
                                nc.tensor.matmul(ps_var, ones_b[:], sq[m][:],
                                                 start=(m == 0), stop=(m == NR - 1))
                            std = p2.tile([1, TPC], F32, tag="std")
                            nc.scalar.activation(std[:], ps_var, AF.Sqrt,
                                                 bias=eps_t[:], scale=1.0 / RANK)
                            istd = p2.tile([1, TPC], F32, tag="istd")
                            nc.vector.reciprocal(istd[:], std[:])
                            istd_bc = p2.tile([128, TPC], F32, tag="ibc")
                            nc.gpsimd.partition_broadcast(istd_bc[:], istd[:])
                            latn = p2.tile([128, NR * TPC], BF)
                            for m in range(NR):
                                nc.vector.tensor_mul(
                                    latn[:, m * TPC:(m + 1) * TPC],
                                    lat_sb[:, m * TPC:(m + 1) * TPC],
                                    istd_bc[:])
                        if g in (1, 2):
                            # latent AllGather split across group ends to
                            # spread the DMA burst
                            h = g - 1
                            for m2 in range(2):
                                m = 2 * h + m2
                                nc.sync.dma_start(
                                    ag_in[h][m2 * 128:(m2 + 1) * 128, :],
                                    latn[:, m * TPC:(m + 1) * TPC])
                            if sim_mode:
                                for s8 in range(NCORES):
                                    nc.sync.dma_start(ag_out[h][s8], ag_in[h][:])
                            else:
                                nc.gpsimd.collective_compute(
                                    "AllGather", mybir.AluOpType.bypass,
                                    replica_groups=RG,
                                    ins=[ag_in[h].opt()], outs=[ag_out[h].opt()])
                        if g == 2:
                            nc.sync.dma_start(
                                kvbT_sb[:].rearrange("p (r m) -> p r m", r=NR),
                                kvbT_in.rearrange("(r p) m -> p r m", p=128))

                # ---------- P3: kv_b for my 2 heads over all tokens
                # kvbT_sb col order per rank chunk r: k0,k1,v0,v1 (128 each).
                with ExitStack() as c3:
                    p3 = c3.enter_context(tc.tile_pool(name=f"p3_{p_}", bufs=1))
                    p3ps = c3.enter_context(tc.tile_pool(name=f"p3ps_{p_}", bufs=1, space="PSUM"))
                    W = HPC * 2 * D               # 512 cols per rank chunk
                    for tc8 in range(NCORES):     # 512-token chunks (AG layout)
                        b, loc = tc8 // 4, (tc8 % 4) * 512
                        lt = [p3.tile([128, 2 * 512], BF, tag=f"lt{h}", bufs=6,
                                      name=f"lth{h}") for h in range(2)]
                        for h in range(2):
                            nc.sync.dma_start(
                                lt[h][:].rearrange("p (r t) -> p r t", r=2),
                                ag_out[h][tc8].rearrange("(r p) t -> p r t", p=128))
                        for j in range(HPC):      # k for head j: [d, tok]
                            ps = p3ps.tile([128, 512], F32, tag="kv", bufs=4)
                            for r in range(NR):
                                nc.tensor.matmul(
                                    ps[:],
                                    kvbT_sb[:, r * W + j * 128:r * W + (j + 1) * 128],
                                    lt[r // 2][:, (r % 2) * 512:(r % 2 + 1) * 512],
                                    start=(r == 0), stop=(r == NR - 1))
                            nc.any.tensor_copy(kT[j][b][:, loc:loc + 512], ps[:])
                        for q4 in range(4):       # v for both heads: [tok, d]
                            ps = p3ps.tile([128, 256], F32, tag="vt", bufs=4)
                            for r in range(NR):
                                nc.tensor.matmul(
                                    ps[:],
                                    lt[r // 2][:, (r % 2) * 512 + q4 * 128:
                                               (r % 2) * 512 + (q4 + 1) * 128],
                                    kvbT_sb[:, r * W + 256:r * W + 512],
                                    start=(r == 0), stop=(r == NR - 1))
                            for j in range(HPC):
                                nc.any.tensor_copy(
                                    vnat[j][b][:, loc + q4 * 128:loc + (q4 + 1) * 128],
                                    ps[:, j * 128:(j + 1) * 128])
                        if tc8 % 4 == 3:
                            for j in range(HPC):
                                rope_block(kT[j][b])

                # ---------- P5: attention per (batch, head), scoresT layout
                with ExitStack() as c5:
                    p5 = c5.enter_context(tc.tile_pool(name=f"p5_{p_}", bufs=1))
                    p5ps = c5.enter_context(tc.tile_pool(name=f"p5ps_{p_}", bufs=1, space="PSUM"))
                    NKT = S // 128   # 16 k-chunks per batch
                    for b in range(B):
                        for j in range(HPC):
                            qt, kt, vn = qT[j][b], kT[j][b], vnat[j][b]
                            for qc in range(4):
                                if b == 0:
                                    # o_proj weight prefetch, paced to
                                    # attention progress: a Pool-engine stamp
                                    # into the dest gives each chunk's DMA a
                                    # WAR wait on this block's Pool work
                                    for o2 in range(2):
                                        om = (j * 4 + qc) * 2 + o2
                                        dst = owt_sb[:, om * HID:(om + 1) * HID]
                                        nc.gpsimd.tensor_copy(
                                            owt_sb[0:1, om * HID:om * HID + 1],
                                            ones_b[0:1, 0:1])
                                        nc.sync.dma_start(dst, owt_in[om])
                                qs = qt[:, qc * 512:(qc + 1) * 512]
                                # created after the first ps_s so the "s" tag
                                # gets the low PSUM banks (freed earliest, so
                                # o_proj's first bank reuse doesn't wait on the
                                # last block's ao chain)
                                ps_av = ps_den = None
                                state = {"s1": None}

                                def av_and_den(kp, e):
                                    for h2 in range(2):
                                        k16 = 2 * kp + h2
                                        es = e[:, h2 * 512:(h2 + 1) * 512]
                                        nc.tensor.matmul(
                                            ps_av[:],
                                            vn[:, k16 * 128:(k16 + 1) * 128], es,
                                            start=(k16 == 0), stop=(k16 == NKT - 1))
                                    s1 = p5.tile([128, 512], BF, tag="s1", bufs=6)
                                    nc.vector.tensor_add(s1[:], e[:, 0:512],
                                                         e[:, 512:1024])
                                    if kp % 2 == 0:
                                        state["s1"] = s1
                                    else:
                                        s2 = p5.tile([128, 512], BF, tag="s2",
                                                     bufs=2)
                                        nc.vector.tensor_add(s2[:], state["s1"][:],
                                                             s1[:])
                                        nc.tensor.matmul(
                                            ps_den[:], ones_b[:], s2[:],
                                            start=(kp == 1), stop=(kp == NKT // 2 - 1))

                                e_prev = None
                                for kp in range(NKT // 2):
                                    ps_s = p5ps.tile([128, 1024], F32, tag="s", bufs=2)
                                    if ps_av is None:
                                        ps_av = p5ps.tile([128, 512], F32,
                                                          tag="av", bufs=2)
                                        ps_den = p5ps.tile([1, 512], F32,
                                                           tag="den", bufs=2)
                                    for h2 in range(2):
                                        k16 = 2 * kp + h2
                                        nc.tensor.matmul(
                                            ps_s[:, h2 * 512:(h2 + 1) * 512],
                                            kt[:, k16 * 128:(k16 + 1) * 128], qs,
                                            start=True, stop=True)
                                    e = p5.tile([128, 1024], BF, tag="e", bufs=8)
                                    nc.scalar.activation(e[:], ps_s[:], AF.Exp)
                                    if e_prev is not None:
                                        av_and_den(kp - 1, e_prev)
                                    e_prev = e
                                av_and_den(NKT // 2 - 1, e_prev)
                                rec = p5.tile([1, 512], F32, tag="rec", bufs=4)
                                nc.vector.reciprocal(rec[:], ps_den[:])
                                rec_bc = p5.tile([128, 512], F32, tag="rbc", bufs=3)
                                nc.gpsimd.partition_broadcast(rec_bc[:], rec[:])
                                ao_t = p5.tile([128, 512], BF, tag="aot", bufs=6)
                                nc.vector.tensor_mul(ao_t[:], ps_av[:], rec_bc[:])
                                for h2a in range(2):
                                    s8 = 2 * qc + h2a
                                    nc.sync.dma_start(
                                        a2a_in[b][s8, j * D:(j + 1) * D, :],
                                        ao_t[:, h2a * SPC:(h2a + 1) * SPC])
                        # AllToAll for this batch as soon as both heads are done
                        if sim_mode:
                            nc.sync.dma_start(a2a_out[b][:], a2a_in[b][:])
                        else:
                            nc.gpsimd.collective_compute(
                                "AllToAll", mybir.AluOpType.bypass, replica_groups=RG,
                                ins=[a2a_in[b].opt()], outs=[a2a_out[b].opt()])
                        if b == 0:
                            for k16 in range(NH):
                                i, halfk = k16 // 2, k16 % 2
                                nc.sync.dma_start(
                                    af[:, k16 * 2 * SPC:k16 * 2 * SPC + SPC],
                                    a2a_out[0][i, halfk * 128:(halfk + 1) * 128, :])

            # ---------- P7: o_proj on my 512 tokens, batch-split
            with ExitStack() as c7:
                p7 = c7.enter_context(tc.tile_pool(name=f"p7_{p_}", bufs=1))
                p7ps = c7.enter_context(tc.tile_pool(name=f"p7ps_{p_}", bufs=1, space="PSUM"))

                def o_proj(b):
                    for om in range(NH):
                        ps_o = p7ps.tile([128, SPC], F32, tag="o", bufs=4)
                        for k16 in range(NH):
                            nc.tensor.matmul(
                                ps_o[:],
                                owt_sb[:, om * HID + k16 * 128:
                                       om * HID + (k16 + 1) * 128],
                                af[:, k16 * 2 * SPC + b * SPC:
                                   k16 * 2 * SPC + (b + 1) * SPC],
                                start=(k16 == 0), stop=(k16 == NH - 1))
                        o_sb = p7.tile([128, SPC], F32, tag="osb", bufs=3)
                        nc.any.tensor_copy(o_sb[:], ps_o[:])
                        nc.sync.dma_start(
                            outTs[p_][om * 128:(om + 1) * 128,
                                      b * SPC:(b + 1) * SPC], o_sb[:])

                o_proj(0)
                for k16 in range(NH):
                    i, halfk = k16 // 2, k16 % 2
                    nc.sync.dma_start(
                        af[:, k16 * 2 * SPC + SPC:(k16 + 1) * 2 * SPC],
                        a2a_out[1][i, halfk * 128:(halfk + 1) * 128, :])
                o_proj(1)
            ctx_pass.close()

    nc.compile()
    return nc


def build_in_maps(hidden_states, q_w, kv_a_w, kv_b_w, o_w, kv_norm_w):
    import ml_dtypes
    BF = ml_dtypes.bfloat16

    hs = np.ascontiguousarray(np.asarray(hidden_states, dtype=np.float32))
    q_w = np.asarray(q_w, dtype=np.float32)
    kv_a_w = np.asarray(kv_a_w, dtype=np.float32)
    kv_b_w = np.asarray(kv_b_w, dtype=np.float32)
    o_w = np.asarray(o_w, dtype=np.float32)
    kv_norm_w = np.asarray(kv_norm_w, dtype=np.float32)

    hsT = np.ascontiguousarray(hs.reshape(T, HID).T).astype(BF)       # [HID, T]
    kvaT = np.ascontiguousarray(kv_a_w[ROPE:, :].T).astype(BF)        # [HID, RANK]
    scale = D ** -0.5
    cosT, sinT = _yarn_cos_sin()
    sinsh = np.concatenate([sinT[32:64], -sinT[0:32]], axis=0).astype(BF)
    cosT = cosT.astype(BF)
    ones_b = np.ones((128, 1), dtype=BF)
    # owt[om, p, k*128+m] = o_w[om*128+m, k*128+p]
    owt = np.ascontiguousarray(
        o_w.reshape(16, 128, 16, 128).transpose(0, 3, 2, 1).reshape(16, 128, HID)
    ).astype(BF)

    kvb = (kv_b_w * kv_norm_w[None, :]).reshape(H, 2, D, RANK)

    in_maps = []
    for c in range(NCORES):
        qwT = np.ascontiguousarray(
            (q_w[c * HPC * D:(c + 1) * HPC * D] * scale).T).astype(BF)  # [HID, 256]
        # kvbT col order per core: k0,k1,v0,v1 (each 128 wide)
        blk = kvb[c * HPC:(c + 1) * HPC]                # [2(head),2(kv),128,RANK]
        blk = blk.transpose(1, 0, 2, 3)                 # [2(kv),2(head),128,RANK]
        kvbT = np.ascontiguousarray(
            blk.reshape(HPC * 2 * D, RANK).T).astype(BF)               # [RANK, 512]
        hsmy = np.ascontiguousarray(hsT[:, c * TPC:(c + 1) * TPC])
        in_maps.append({
            "hsT": hsT, "hsmy": hsmy, "qwT": qwT, "kvaT": kvaT,
            "kvbT": kvbT, "owt": owt, "cos": cosT, "sinsh": sinsh,
            "onesb": ones_b,
        })
    return in_maps


def assemble_output(results):
    out = np.empty((B, S, HID), dtype=np.float32)
    for c in range(NCORES):
        r = results[c]["outT"]                 # [HID, 2*SPC]
        out[0, c * SPC:(c + 1) * SPC, :] = r[:, 0:SPC].T
        out[1, c * SPC:(c + 1) * SPC, :] = r[:, SPC:2 * SPC].T
    return out


def kernel(hidden_states, q_w, kv_a_w, kv_b_w, o_w, kv_norm_w):
    from concourse import bass_utils

    if "nc" not in _CACHE:
        _CACHE["nc"] = build_nc()
    nc = _CACHE["nc"]
    in_maps = build_in_maps(hidden_states, q_w, kv_a_w, kv_b_w, o_w, kv_norm_w)
    res = bass_utils.run_bass_kernel_spmd(
        nc, in_maps, core_ids=list(range(NCORES)), trace=False)
    return assemble_output(res.results)


# revision 21
# speedup vs baseline: 3.9793x; 3.9793x over previous
"""MultiHeadLatentAttention on 8 Trainium2 NeuronCores (Bass/Tile, SPMD).

Sharding (tensor parallel over heads, per the hint, plus refinements):
  - 16 heads / 8 cores = 2 heads per core: q_proj + kv_b_proj output dims and
    o_proj input dim sharded by head.
  - kv_a_proj + rms-norm are token-sharded (512 tokens/core) with an
    AllGather of the normalized latent instead of replicating the kv_a matmul.
  - AllToAll of the attention outputs token-shards the o_proj: each core
    computes the full o_proj for 512 tokens and outputs its token slice.

Perf notes (v2, ~584us -> ~345us single-pass):
  - All HBM-resident operands are bf16 (same 1 cycle/row PE rate as fp32r,
    half the DMA bytes); all matmul accumulation is fp32 in PSUM.
  - kv_a rides inside q_proj's first token-group loop (shared DMA stream,
    PSUM = 4 latent + 4 q banks); the rms-norm epilogue runs after group 1
    and the latent AllGather issues behind group 1/2 tiles, so the PE never
    waits on the norm chain or the collective.
  - Softmax denominator: exp chunks are pair/quad-summed on the DVE (bf16,
    2x mode), so the PE does only 4 ones-matmuls per 512-token block
    instead of 16.  Partition broadcasts (1/den, 1/std) run on the idle
    GpSimd engine instead of PE ones-matmuls.
  - V is produced directly in [tok, d] layout (latent as stationary), so
    the per-chunk PE transposes are gone.
  - o_proj is split by batch; its weights prefetch during batch-0 attention,
    each chunk's DMA paced by a Pool-engine stamp so the SP's run-ahead
    cannot flood the queues; o_proj(b0) covers the batch-1 AllToAll.
  - Attention phase is Activation-bound (exp at ~1us per [128,1024] tile);
    scores(kp+1) are issued before av(kp) so the PE pipelines around exp.
"""

import math
from contextlib import ExitStack

import numpy as np

B, S = 2, 2048
T = B * S                     # 4096 flattened tokens
HID = 2048
H, D = 16, 128
RANK, ROPE = 512, 64
MAX_POS, ORIG_POS = 131072, 8192
BASE = 500000.0
BETA_FAST, BETA_SLOW = 32.0, 1.0
EPS = 1e-6
NCORES = 8
HPC = H // NCORES             # 2 heads per core
TPC = T // NCORES             # 512 tokens per core (kv_a shard)
SPC = S // NCORES             # 256 tokens per (core, batch) after AllToAll

_CACHE: dict = {}


def _yarn_cos_sin():
    """cos/sin tables matching reference.py's yarn_cos_sin (mscale folded)."""
    scaling = MAX_POS / ORIG_POS
    pos_freqs = BASE ** (np.arange(0, ROPE, 2, dtype=np.float64) / ROPE)
    extrap = 1.0 / pos_freqs
    interp = 1.0 / (scaling * pos_freqs)
    low = max(math.floor(ROPE * math.log(ORIG_POS / (BETA_FAST * 2 * math.pi))
                         / (2 * math.log(BASE))), 0)
    high = min(math.ceil(ROPE * math.log(ORIG_POS / (BETA_SLOW * 2 * math.pi))
                         / (2 * math.log(BASE))), ROPE - 1)
    i = np.arange(ROPE // 2, dtype=np.float64)
    smooth = np.clip((i - low) / max(high - low, 1), 0.0, 1.0)
    inv_freq = ((1.0 - smooth) * interp + smooth * extrap).astype(np.float32)
    pos = np.arange(S, dtype=np.float32)
    freqs = pos[:, None] * inv_freq[None, :]              # [S, 32]
    emb = np.concatenate([freqs, freqs], axis=-1)         # [S, 64]
    mscale = 0.1 * math.log(scaling) + 1.0
    cos = (np.cos(emb) * mscale).astype(np.float32)
    sin = (np.sin(emb) * mscale).astype(np.float32)
    return cos.T.copy(), sin.T.copy()                     # [64, S] each


def build_nc(passes=1, sim_mode=False):
    """Build + compile the (single, SPMD) Bass program for all 8 cores."""
    import concourse.tile as tile
    import concourse.mybir as mybir
    from concourse import bacc

    F32 = mybir.dt.float32
    BF = mybir.dt.bfloat16
    AF = mybir.ActivationFunctionType
    RG = [list(range(NCORES))]

    nc = bacc.Bacc("TRN2", target_bir_lowering=False, debug=False,
                   num_devices=1 if sim_mode else NCORES)

    # ---- kernel I/O ----
    hsT_in = nc.dram_tensor("hsT", [HID, T], BF, kind="ExternalInput").ap()
    hsmy_in = nc.dram_tensor("hsmy", [HID, TPC], BF, kind="ExternalInput").ap()
    qwT_in = nc.dram_tensor("qwT", [HID, HPC * D], BF, kind="ExternalInput").ap()
    kvaT_in = nc.dram_tensor("kvaT", [HID, RANK], BF, kind="ExternalInput").ap()
    kvbT_in = nc.dram_tensor("kvbT", [RANK, HPC * 2 * D], BF, kind="ExternalInput").ap()
    owt_in = nc.dram_tensor("owt", [16, 128, HID], BF, kind="ExternalInput").ap()
    cos_in = nc.dram_tensor("cos", [ROPE, S], BF, kind="ExternalInput").ap()
    sinsh_in = nc.dram_tensor("sinsh", [ROPE, S], BF, kind="ExternalInput").ap()
    onesb_in = nc.dram_tensor("onesb", [128, 1], BF, kind="ExternalInput").ap()
    outTs = [nc.dram_tensor(f"outT{p}" if p else "outT", [HID, 2 * SPC], F32,
                            kind="ExternalOutput").ap() for p in range(passes)]

    NH = HID // 128   # 16 hid chunks
    NR = RANK // 128  # 4 rank chunks

    with tile.TileContext(nc) as tc, ExitStack() as ctx0:
        const = ctx0.enter_context(tc.tile_pool(name="const", bufs=1))
        dram = ctx0.enter_context(tc.tile_pool(name="dram", bufs=1, space="DRAM"))

        ones_b = const.tile([128, 1], BF)
        cosb = const.tile([ROPE, S], BF)
        sinsh = const.tile([ROPE, S], BF)
        eps_t = const.tile([1, 1], F32)
        nc.sync.dma_start(ones_b[:], onesb_in[:])
        nc.vector.memset(eps_t[:], EPS)

        for p_ in range(passes):
            # collective bounce buffers
            ag_in = [dram.tile([RANK // 2, TPC], BF, name=f"agin{p_}{h}")
                     for h in range(2)]
            ag_out = [dram.tile([NCORES, RANK // 2, TPC], BF,
                                addr_space="Local" if sim_mode else "Shared",
                                name=f"agout{p_}{h}") for h in range(2)]
            a2a_in = [dram.tile([NCORES, HPC * D, SPC], BF, name=f"a2ain{p_}{b}")
                      for b in range(B)]
            a2a_out = [dram.tile([NCORES, HPC * D, SPC], BF, name=f"a2aout{p_}{b}")
                       for b in range(B)]

            ctx_pass = ExitStack()
            afp = ctx_pass.enter_context(tc.tile_pool(name=f"afp_{p_}", bufs=1))
            af = afp.tile([128, NH * 2 * SPC], BF, name=f"af{p_}")
            owt_sb = afp.tile([128, NH * HID], BF, name=f"owt{p_}")
            qw_sb = afp.tile([128, NH * HPC * D], BF, name=f"qw{p_}")
            kvbT_sb = afp.tile([128, NR * HPC * 2 * D], BF, name=f"kvb{p_}")
            with ExitStack() as ctx_big:
                big = ctx_big.enter_context(tc.tile_pool(name=f"big_{p_}", bufs=1))
                rope_pool = ctx_big.enter_context(
                    tc.tile_pool(name=f"rope_{p_}", bufs=1))

                def rope_block(X):
                    tmp = rope_pool.tile([ROPE, S], BF, tag="rtmp", bufs=1,
                                         name="rtmp")
                    m2 = rope_pool.tile([ROPE, S], BF, tag="rm2", bufs=1,
                                        name="rm2")
                    nc.vector.tensor_mul(tmp[:], X[0:ROPE], cosb[:])
                    nc.vector.tensor_mul(m2[0:32], X[32:64], sinsh[32:64])
                    nc.vector.tensor_mul(m2[32:64], X[0:32], sinsh[0:32])
                    nc.vector.tensor_add(X[0:ROPE], tmp[:], m2[:])

                # per (head j, batch b) tiles, [128, S] each
                qT = [[big.tile([128, S], BF, name=f"qT{p_}{j}{b}") for b in range(B)]
                      for j in range(HPC)]
                kT = [[big.tile([128, S], BF, name=f"kT{p_}{j}{b}") for b in range(B)]
                      for j in range(HPC)]
                vnat = [[big.tile([128, S], BF, name=f"vn{p_}{j}{b}") for b in range(B)]
                        for j in range(HPC)]

                # ---------- P1+P2 fused: q_proj over 4 token groups; kv_a on my
                # 512-token shard rides along with group 0 (shared DMA stream);
                # the rms-norm epilogue + AllGather are deferred to after
                # group 1 so the PE never waits on DVE/Act.
                with ExitStack() as c2:
                    p2 = c2.enter_context(tc.tile_pool(name=f"p2_{p_}", bufs=1))
                    p2ps = c2.enter_context(tc.tile_pool(name=f"p2ps_{p_}", bufs=1, space="PSUM"))
                    ps_lat = [p2ps.tile([128, TPC], F32, name=f"pslat{p_}{m}", tag=f"lat{m}")
                              for m in range(NR)]
                    lat_sb = p2.tile([128, NR * TPC], F32)

                    for g in range(4):            # 1024-token groups
                        b, half = g // 2, g % 2
                        psq = [[p2ps.tile([128, 512], F32, name=f"psq{p_}{g}{m}{t2}",
                                          tag="psq", bufs=4)
                                for t2 in range(2)] for m in range(HPC)]
                        for k in range(NH):
                            if g == 0:
                                nc.sync.dma_start(
                                    qw_sb[:, k * HPC * D:(k + 1) * HPC * D],
                                    qwT_in[k * 128:(k + 1) * 128, :])
                            ht = p2.tile([128, 1024], BF, tag="hsq", bufs=12)
                            nc.sync.dma_start(
                                ht[:], hsT_in[k * 128:(k + 1) * 128,
                                              g * 1024:(g + 1) * 1024])
                            if g == 0:
                                kva_t = p2.tile([128, RANK], BF, tag="kvat", bufs=3)
                                nc.sync.dma_start(
                                    kva_t[:], kvaT_in[k * 128:(k + 1) * 128, :])
                                hm = p2.tile([128, TPC], BF, tag="hsmy", bufs=3)
                                nc.sync.dma_start(
                                    hm[:], hsmy_in[k * 128:(k + 1) * 128, :])
                            for m in range(HPC):
                                for t2 in range(2):
                                    nc.tensor.matmul(
                                        psq[m][t2][:],
                                        qw_sb[:, k * HPC * D + m * 128:
                                              k * HPC * D + (m + 1) * 128],
                                        ht[:, t2 * 512:(t2 + 1) * 512],
                                        start=(k == 0), stop=(k == NH - 1))
                            if g == 0:
                                for m in range(NR):
                                    nc.tensor.matmul(
                                        ps_lat[m][:],
                                        kva_t[:, m * 128:(m + 1) * 128],
                                        hm[:], start=(k == 0), stop=(k == NH - 1))
                        for m in range(HPC):
                            for t2 in range(2):
                                col = half * 1024 + t2 * 512
                                nc.any.tensor_copy(qT[m][b][:, col:col + 512],
                                                   psq[m][t2][:])
                        if half == 1:
                            for j in range(HPC):
                                rope_block(qT[j][b])
                        if g == 0:
                            # drain the latent out of PSUM; square on DVE while
                            # group 1 streams
                            for m in range(NR):
                                nc.any.tensor_copy(
                                    lat_sb[:, m * TPC:(m + 1) * TPC], ps_lat[m][:])
                            sq = [p2.tile([128, TPC], BF, name=f"sq{p_}{m}",
                                          tag="sq", bufs=2) for m in range(NR)]
                            for m in range(NR):
                                nc.vector.tensor_mul(
                                    sq[m][:], lat_sb[:, m * TPC:(m + 1) * TPC],
                                    lat_sb[:, m * TPC:(m + 1) * TPC])
                            nc.sync.dma_start(cosb[:], cos_in[:])
                            nc.sync.dma_start(sinsh[:], sinsh_in[:])
                        if g == 1:
                            # rms-norm stats; var accumulates into a corner
                            # of the (drained) ps_lat[0] bank
                            ps_var = ps_lat[0][0:1, :]
                            for m in range(NR):
                                nc.tensor.matmul(ps_var, ones_b[:], sq[m][:],
                                                 start=(m == 0), stop=(m == NR - 1))
                            std = p2.tile([1, TPC], F32, tag="std")
                            nc.scalar.activation(std[:], ps_var, AF.Sqrt,
                                                 bias=eps_t[:], scale=1.0 / RANK)
                            istd = p2.tile([1, TPC], F32, tag="istd")
                            nc.vector.reciprocal(istd[:], std[:])
                            istd_bc = p2.tile([128, TPC], F32, tag="ibc")
                            nc.gpsimd.partition_broadcast(istd_bc[:], istd[:])
                            latn = p2.tile([128, NR * TPC], BF)
                            for m in range(NR):
                                nc.vector.tensor_mul(
                                    latn[:, m * TPC:(m + 1) * TPC],
                                    lat_sb[:, m * TPC:(m + 1) * TPC],
                                    istd_bc[:])
                        if g in (1, 2):
                            # latent AllGather split across group ends to
                            # spread the DMA burst
                            h = g - 1
                            for m2 in range(2):
                                m = 2 * h + m2
                                nc.sync.dma_start(
                                    ag_in[h][m2 * 128:(m2 + 1) * 128, :],
                                    latn[:, m * TPC:(m + 1) * TPC])
                            if sim_mode:
                                for s8 in range(NCORES):
                                    nc.sync.dma_start(ag_out[h][s8], ag_in[h][:])
                            else:
                                nc.gpsimd.collective_compute(
                                    "AllGather", mybir.AluOpType.bypass,
                                    replica_groups=RG,
                                    ins=[ag_in[h].opt()], outs=[ag_out[h].opt()])
                        if g == 2:
                            nc.sync.dma_start(
                                kvbT_sb[:].rearrange("p (r m) -> p r m", r=NR),
                                kvbT_in.rearrange("(r p) m -> p r m", p=128))

                # ---------- P3: kv_b for my 2 heads over all tokens
                # kvbT_sb col order per rank chunk r: k0,k1,v0,v1 (128 each).
                with ExitStack() as c3:
                    p3 = c3.enter_context(tc.tile_pool(name=f"p3_{p_}", bufs=1))
                    p3ps = c3.enter_context(tc.tile_pool(name=f"p3ps_{p_}", bufs=1, space="PSUM"))
                    W = HPC * 2 * D               # 512 cols per rank chunk
                    for tc8 in range(NCORES):     # 512-token chunks (AG layout)
                        b, loc = tc8 // 4, (tc8 % 4) * 512
                        lt = [p3.tile([128, 2 * 512], BF, tag=f"lt{h}", bufs=6,
                                      name=f"lth{h}") for h in range(2)]
                        for h in range(2):
                            nc.sync.dma_start(
                                lt[h][:].rearrange("p (r t) -> p r t", r=2),
                                ag_out[h][tc8].rearrange("(r p) t -> p r t", p=128))
                        for j in range(HPC):      # k for head j: [d, tok]
                            ps = p3ps.tile([128, 512], F32, tag="kv", bufs=4)
                            for r in range(NR):
                                nc.tensor.matmul(
                                    ps[:],
                                    kvbT_sb[:, r * W + j * 128:r * W + (j + 1) * 128],
                                    lt[r // 2][:, (r % 2) * 512:(r % 2 + 1) * 512],
                                    start=(r == 0), stop=(r == NR - 1))
                            nc.any.tensor_copy(kT[j][b][:, loc:loc + 512], ps[:])
                        for q4 in range(4):       # v for both heads: [tok, d]
                            ps = p3ps.tile([128, 256], F32, tag="vt", bufs=4)
                            for r in range(NR):
                                nc.tensor.matmul(
                                    ps[:],
                                    lt[r // 2][:, (r % 2) * 512 + q4 * 128:
                                               (r % 2) * 512 + (q4 + 1) * 128],
                                    kvbT_sb[:, r * W + 256:r * W + 512],
                                    start=(r == 0), stop=(r == NR - 1))
                            for j in range(HPC):
                                nc.any.tensor_copy(
                                    vnat[j][b][:, loc + q4 * 128:loc + (q4 + 1) * 128],
                                    ps[:, j * 128:(j + 1) * 128])
                        if tc8 % 4 == 3:
                            for j in range(HPC):
                                rope_block(kT[j][b])

                # ---------- P5: attention per (batch, head), scoresT layout
                with ExitStack() as c5:
                    p5 = c5.enter_context(tc.tile_pool(name=f"p5_{p_}", bufs=1))
                    p5ps = c5.enter_context(tc.tile_pool(name=f"p5ps_{p_}", bufs=1, space="PSUM"))
                    NKT = S // 128   # 16 k-chunks per batch
                    for b in range(B):
                        for j in range(HPC):
                            qt, kt, vn = qT[j][b], kT[j][b], vnat[j][b]
                            for qc in range(4):
                                if b == 0:
                                    # o_proj weight prefetch, paced to
                                    # attention progress: a Pool-engine stamp
                                    # into the dest gives each chunk's DMA a
                                    # WAR wait on this block's Pool work
                                    for o2 in range(2):
                                        om = (j * 4 + qc) * 2 + o2
                                        dst = owt_sb[:, om * HID:(om + 1) * HID]
                                        nc.gpsimd.tensor_copy(
                                            owt_sb[0:1, om * HID:om * HID + 1],
                                            ones_b[0:1, 0:1])
                                        nc.sync.dma_start(dst, owt_in[om])
                                qs = qt[:, qc * 512:(qc + 1) * 512]
                                # created after the first ps_s so the "s" tag
                                # gets the low PSUM banks (freed earliest, so
                                # o_proj's first bank reuse doesn't wait on the
                                # last block's ao chain)
                                ps_av = ps_den = None
                                state = {"s1": None}

                                def av_and_den(kp, e):
                                    for h2 in range(2):
                                        k16 = 2 * kp + h2
                                        es = e[:, h2 * 512:(h2 + 1) * 512]
                                        nc.tensor.matmul(
                                            ps_av[:],
                                            vn[:, k16 * 128:(k16 + 1) * 128], es,
                                            start=(k16 == 0), stop=(k16 == NKT - 1))
                                    s1 = p5.tile([128, 512], BF, tag="s1", bufs=6)
                                    nc.vector.tensor_add(s1[:], e[:, 0:512],
                                                         e[:, 512:1024])
                                    if kp % 2 == 0:
                                        state["s1"] = s1
                                    else:
                                        s2 = p5.tile([128, 512], BF, tag="s2",
                                                     bufs=2)
                                        nc.vector.tensor_add(s2[:], state["s1"][:],
                                                             s1[:])
                                        nc.tensor.matmul(
                                            ps_den[:], ones_b[:], s2[:],
                                            start=(kp == 1), stop=(kp == NKT // 2 - 1))

                                e_prev = None
                                for kp in range(NKT // 2):
                                    ps_s = p5ps.tile([128, 1024], F32, tag="s", bufs=2)
                                    if ps_av is None:
                                        ps_av = p5ps.tile([128, 512], F32,
                                                          tag="av", bufs=2)
                                        ps_den = p5ps.tile([1, 512], F32,
                                                           tag="den", bufs=2)
                                    for h2 in range(2):
                                        k16 = 2 * kp + h2
                                        nc.tensor.matmul(
                                            ps_s[:, h2 * 512:(h2 + 1) * 512],
                                            kt[:, k16 * 128:(k16 + 1) * 128], qs,
                                            start=True, stop=True)
                                    e = p5.tile([128, 1024], BF, tag="e", bufs=8)
                                    nc.scalar.activation(e[:], ps_s[:], AF.Exp)
                                    if e_prev is not None:
                                        av_and_den(kp - 1, e_prev)
                                    e_prev = e
                                av_and_den(NKT // 2 - 1, e_prev)
                                rec = p5.tile([1, 512], F32, tag="rec", bufs=4)
                                nc.vector.reciprocal(rec[:], ps_den[:])
                                rec_bc = p5.tile([128, 512], F32, tag="rbc", bufs=3)
                                nc.gpsimd.partition_broadcast(rec_bc[:], rec[:])
                                ao_t = p5.tile([128, 512], BF, tag="aot", bufs=6)
                                nc.vector.tensor_mul(ao_t[:], ps_av[:], rec_bc[:])
                                for h2a in range(2):
                                    s8 = 2 * qc + h2a
                                    nc.sync.dma_start(
                                        a2a_in[b][s8, j * D:(j + 1) * D, :],
                                        ao_t[:, h2a * SPC:(h2a + 1) * SPC])
                        # AllToAll for this batch as soon as both heads are done
                        if sim_mode:
                            nc.sync.dma_start(a2a_out[b][:], a2a_in[b][:])
                        else:
                            nc.gpsimd.collective_compute(
                                "AllToAll", mybir.AluOpType.bypass, replica_groups=RG,
                                ins=[a2a_in[b].opt()], outs=[a2a_out[b].opt()])
                        if b == 0:
                            for k16 in range(NH):
                                i, halfk = k16 // 2, k16 % 2
                                nc.sync.dma_start(
                                    af[:, k16 * 2 * SPC:k16 * 2 * SPC + SPC],
                                    a2a_out[0][i, halfk * 128:(halfk + 1) * 128, :])

            # ---------- P7: o_proj on my 512 tokens, batch-split
            with ExitStack() as c7:
                p7 = c7.enter_context(tc.tile_pool(name=f"p7_{p_}", bufs=1))
                p7ps = c7.enter_context(tc.tile_pool(name=f"p7ps_{p_}", bufs=1, space="PSUM"))

                def o_proj(b):
                    for om in range(NH):
                        ps_o = p7ps.tile([128, SPC], F32, tag="o", bufs=4)
                        for k16 in range(NH):
                            nc.tensor.matmul(
                                ps_o[:],
                                owt_sb[:, om * HID + k16 * 128:
                                       om * HID + (k16 + 1) * 128],
                                af[:, k16 * 2 * SPC + b * SPC:
                                   k16 * 2 * SPC + (b + 1) * SPC],
                                start=(k16 == 0), stop=(k16 == NH - 1))
                        o_sb = p7.tile([128, SPC], F32, tag="osb", bufs=3)
                        nc.any.tensor_copy(o_sb[:], ps_o[:])
                        nc.sync.dma_start(
                            outTs[p_][om * 128:(om + 1) * 128,
                                      b * SPC:(b + 1) * SPC], o_sb[:])

                o_proj(0)
                for k16 in range(NH):
                    i, halfk = k16 // 2, k16 % 2
                    nc.sync.dma_start(
                        af[:, k16 * 2 * SPC + SPC:(k16 + 1) * 2 * SPC],
                        a2a_out[1][i, halfk * 128:(halfk + 1) * 128, :])
                o_proj(1)
            ctx_pass.close()

    nc.compile()
    return nc


def build_in_maps(hidden_states, q_w, kv_a_w, kv_b_w, o_w, kv_norm_w):
    import ml_dtypes
    BF = ml_dtypes.bfloat16

    hs = np.ascontiguousarray(np.asarray(hidden_states, dtype=np.float32))
    q_w = np.asarray(q_w, dtype=np.float32)
    kv_a_w = np.asarray(kv_a_w, dtype=np.float32)
    kv_b_w = np.asarray(kv_b_w, dtype=np.float32)
    o_w = np.asarray(o_w, dtype=np.float32)
    kv_norm_w = np.asarray(kv_norm_w, dtype=np.float32)

    hsT = np.ascontiguousarray(hs.reshape(T, HID).T).astype(BF)       # [HID, T]
    kvaT = np.ascontiguousarray(kv_a_w[ROPE:, :].T).astype(BF)        # [HID, RANK]
    scale = D ** -0.5
    cosT, sinT = _yarn_cos_sin()
    sinsh = np.concatenate([sinT[32:64], -sinT[0:32]], axis=0).astype(BF)
    cosT = cosT.astype(BF)
    ones_b = np.ones((128, 1), dtype=BF)
    # owt[om, p, k*128+m] = o_w[om*128+m, k*128+p]
    owt = np.ascontiguousarray(
        o_w.reshape(16, 128, 16, 128).transpose(0, 3, 2, 1).reshape(16, 128, HID)
    ).astype(BF)

    kvb = (kv_b_w * kv_norm_w[None, :]).reshape(H, 2, D, RANK)

    in_maps = []
    for c in range(NCORES):
        qwT = np.ascontiguousarray(
            (q_w[c * HPC * D:(c + 1) * HPC * D] * scale).T).astype(BF)  # [HID, 256]
        # kvbT col order per core: k0,k1,v0,v1 (each 128 wide)
        blk = kvb[c * HPC:(c + 1) * HPC]                # [2(head),2(kv),128,RANK]
        blk = blk.transpose(1, 0, 2, 3)                 # [2(kv),2(head),128,RANK]
        kvbT = np.ascontiguousarray(
            blk.reshape(HPC * 2 * D, RANK).T).astype(BF)               # [RANK, 512]
        hsmy = np.ascontiguousarray(hsT[:, c * TPC:(c + 1) * TPC])
        in_maps.append({
            "hsT": hsT, "hsmy": hsmy, "qwT": qwT, "kvaT": kvaT,
            "kvbT": kvbT, "owt": owt, "cos": cosT, "sinsh": sinsh,
            "onesb": ones_b,
        })
    return in_maps


def assemble_output(results):
    out = np.empty((B, S, HID), dtype=np.float32)
    for c in range(NCORES):
        r = results[c]["outT"]                 # [HID, 2*SPC]
        out[0, c * SPC:(c + 1) * SPC, :] = r[:, 0:SPC].T
        out[1, c * SPC:(c + 1) * SPC, :] = r[:, SPC:2 * SPC].T
    return out


def kernel(hidden_states, q_w, kv_a_w, kv_b_w, o_w, kv_norm_w):
    from concourse import bass_utils

    if "nc" not in _CACHE:
        _CACHE["nc"] = build_nc()
    nc = _CACHE["nc"]
    in_maps = build_in_maps(hidden_states, q_w, kv_a_w, kv_b_w, o_w, kv_norm_w)
    res = bass_utils.run_bass_kernel_spmd(
        nc, in_maps, core_ids=list(range(NCORES)), trace=False)
    return assemble_output(res.results)
